# revision 1
# baseline (speedup 1.0000x reference)
"""Trainium2 Bass kernel for nn_CrossAttention (B=2, S=C=4096, D=512, H=8, Dh=64).

Sharding: batch x head-pair parallel over 8 cores. Core c handles batch
b = c//4 and heads {2*(c%4), 2*(c%4)+1}. Each core computes full attention
for its two heads plus its partial contribution to the output projection;
the host sums the 4 per-core partials per batch and adds the bias.

Device-side dataflow per core (all transposed layouts, no on-chip
transposes needed):
  qT [128=2*dh, S]  = wqT_slice.T @ xT          (f32r matmuls)
  kT [128=2*dh, C]  = wkT_slice.T @ ctxT
  v  [c, 2*dh]      = ctxT.T @ wvT_slice        -> v_aug [c, 65] with ones col
  sT chunk [128c, 512q] = kT_h_chunk.T @ qT_h   (two heads row-tiled on PE)
  P = exp(SCALE * sT)                            (ACT, f32r out)
  o_aug [65, 512q] += v_aug_chunk.T @ P_chunk    (ones col -> row 64 = denom)
  oT = o_aug[0:64] * (1/denom broadcast via K=1 ones matmul)
  y_partial [128s, 512] = sum_h oT_h_chunk.T @ woT_h

Numerics: f32r (tf32) matmuls with host-side pre-rounding of DRAM inputs;
products of tf32 values accumulate exactly in fp32, so the only error is
the tf32 input rounding (~5e-4) plus exp(2 ULP) and the softmax reciprocal
(~51 ULP from reciprocal_approx_fast).
"""

import os
import numpy as np
from contextlib import ExitStack

import concourse.bass as bass
import concourse.tile as tile
from concourse import bacc, mybir
from concourse.bass_utils import run_bass_kernel_spmd

F32 = mybir.dt.float32
F32R = mybir.dt.float32r
EXP = mybir.ActivationFunctionType.Exp

B = 2
S = 4096
C = 4096
D = 512
DH = 64
SCALE = DH ** -0.5  # 0.125

NQB = S // 512   # 8 query blocks of 512
NCB = C // 128   # 32 context chunks of 128
NKC = D // 128   # 4 contraction chunks of 128
NNC = S // 512   # 8 free-dim chunks of 512 for q/k projections
VW = DH + 1      # 65: v_aug chunk width (ones column at 64)

_CACHE = {}


def round_tf32(a: np.ndarray) -> np.ndarray:
    b = np.ascontiguousarray(a, dtype=np.float32).view(np.uint32)
    b = (b + np.uint32(0x1000)) & np.uint32(0xFFFFE000)
    return b.view(np.float32)


def build_nc():
    nc = bacc.Bacc("TRN2", target_bir_lowering=False, debug=False)
    nqb = int(os.environ.get("ATT_QB", NQB))
    rowtile = os.environ.get("ROWTILE", "1") == "1"

    xT = nc.dram_tensor("xT", [D, S], F32R, kind="ExternalInput").ap()
    ctxT = nc.dram_tensor("ctxT", [D, C], F32R, kind="ExternalInput").ap()
    wqT = nc.dram_tensor("wqT", [D, 128], F32R, kind="ExternalInput").ap()
    wkT = nc.dram_tensor("wkT", [D, 128], F32R, kind="ExternalInput").ap()
    wvT = nc.dram_tensor("wvT", [D, 128], F32R, kind="ExternalInput").ap()
    woT = nc.dram_tensor("woT", [128, D], F32R, kind="ExternalInput").ap()
    vones = nc.dram_tensor("vones", [128, NCB], F32R, kind="ExternalInput").ap()
    onesk = nc.dram_tensor("onesk", [1, DH], F32, kind="ExternalInput").ap()
    y = nc.dram_tensor("y", [S, D], F32, kind="ExternalOutput").ap()
    dbg_den = nc.dram_tensor("dbg_den", [1, 512], F32, kind="ExternalOutput").ap()
    dbg_rc = nc.dram_tensor("dbg_rc", [1, 512], F32, kind="ExternalOutput").ap()

    with tile.TileContext(nc) as tc, ExitStack() as ctx:
        sb = ctx.enter_context(tc.tile_pool(name="sb", bufs=1))

        # ---- persistent SBUF tiles ----
        wq_sb = sb.tile([128, D], F32R, name="wq_sb")
        wk_sb = sb.tile([128, D], F32R, name="wk_sb")
        wv_sb = sb.tile([128, D], F32R, name="wv_sb")
        wo0_sb = sb.tile([64, D], F32R, name="wo0_sb")
        wo1_sb = sb.tile([64, D], F32R, name="wo1_sb")
        onesk_sb = sb.tile([1, DH], F32, name="onesk_sb")
        kT_sb = sb.tile([128, C], F32R, name="kT_sb")
        qT_sb = sb.tile([128, S], F32R, name="qT_sb")
        v0_sb = sb.tile([128, NCB * VW], F32R, name="v0_sb")
        v1_sb = sb.tile([128, NCB * VW], F32R, name="v1_sb")

        for kc in range(NKC):
            nc.sync.dma_start(wq_sb[:, kc * 128:(kc + 1) * 128],
                              wqT[kc * 128:(kc + 1) * 128, :])
            nc.sync.dma_start(wk_sb[:, kc * 128:(kc + 1) * 128],
                              wkT[kc * 128:(kc + 1) * 128, :])
            nc.sync.dma_start(wv_sb[:, kc * 128:(kc + 1) * 128],
                              wvT[kc * 128:(kc + 1) * 128, :])
        nc.sync.dma_start(wo0_sb[:], woT[0:64, :])
        nc.sync.dma_start(wo1_sb[:], woT[64:128, :])
        nc.sync.dma_start(onesk_sb[:], onesk)
        # ones columns of v_aug (position 64 of each 65-wide chunk)
        v0_3d = v0_sb.rearrange("p (c k) -> p c k", k=VW)
        v1_3d = v1_sb.rearrange("p (c k) -> p c k", k=VW)
        nc.sync.dma_start(v0_3d[:, :, 64:65], vones.unsqueeze(2))
        nc.sync.dma_start(v1_3d[:, :, 64:65], vones.unsqueeze(2))

        # ---- one shared PSUM pool; proj borrows the bufs=1 slots ----
        with tc.tile_pool(name="aps", bufs=1, space="PSUM") as aps, \
             tc.tile_pool(name="inbig", bufs=10) as inbig, \
             tc.tile_pool(name="psb", bufs=4) as psb, \
             tc.tile_pool(name="msb", bufs=2) as msb:
            # input halves, attention-critical DMAs first
            ctx_ch = [[None] * 2 for _ in range(NKC)]
            x_ch = [[None] * 2 for _ in range(NKC)]
            for h, arr, src_ap, nm in ((0, ctx_ch, ctxT, "ctx"), (0, x_ch, xT, "x"),
                                       (1, ctx_ch, ctxT, "ctx"), (1, x_ch, xT, "x")):
                for kc in range(NKC):
                    t = inbig.tile([128, 2048], F32R, name=f"{nm}{kc}_{h}",
                                   tag="in")
                    nc.sync.dma_start(t[:], src_ap[kc * 128:(kc + 1) * 128,
                                                   h * 2048:(h + 1) * 2048])
                    arr[kc][h] = t

            def kproj(n):
                h = n // 4
                pk = aps.tile([128, 512], F32, name=f"pk{n}", tag="py", bufs=1)
                for kc in range(NKC):
                    nc.tensor.matmul(pk[:], wk_sb[:, kc * 128:(kc + 1) * 128],
                                     ctx_ch[kc][h][:, (n - 4 * h) * 512:
                                                   (n - 4 * h + 1) * 512],
                                     start=(kc == 0), stop=(kc == NKC - 1))
                nc.vector.tensor_copy(kT_sb[:, n * 512:(n + 1) * 512], pk[:])

            def qproj(n):
                h = n // 4
                pq = aps.tile([128, 512], F32, name=f"pq{n}", tag="py", bufs=1)
                for kc in range(NKC):
                    nc.tensor.matmul(pq[:], wq_sb[:, kc * 128:(kc + 1) * 128],
                                     x_ch[kc][h][:, (n - 4 * h) * 512:
                                                 (n - 4 * h + 1) * 512],
                                     start=(kc == 0), stop=(kc == NKC - 1))
                nc.vector.tensor_copy(qT_sb[:, n * 512:(n + 1) * 512], pq[:])

            def vproj(cb):
                h = cb // 16
                pv = aps.tile([128, 128], F32, name=f"pv{cb}", tag="bc", bufs=1)
                for kc in range(NKC):
                    nc.tensor.matmul(pv[:],
                                     ctx_ch[kc][h][:, (cb - 16 * h) * 128:
                                                   (cb - 16 * h + 1) * 128],
                                     wv_sb[:, kc * 128:(kc + 1) * 128],
                                     start=(kc == 0), stop=(kc == NKC - 1))
                nc.vector.tensor_copy(v0_sb[:, cb * VW:cb * VW + DH], pv[:, 0:64])
                nc.vector.tensor_copy(v1_sb[:, cb * VW:cb * VW + DH], pv[:, 64:128])

            for n in range(4):
                kproj(n)
            qproj(0)

            def pre_work(qb, g):
                # software-pipelined remainder of the projections inside qb0
                if qb == 0:
                    if g == 0:
                        for cb in range(6):
                            vproj(cb)
                    elif g <= 13:
                        vproj(2 * g + 4)
                        vproj(2 * g + 5)
                    if 3 <= g <= 6:
                        kproj(g + 1)
                if g == 0 and qb + 1 < NQB:
                    qproj(qb + 1)

            # ---- attention + output projection ----
            for qb in range(nqb):
                qsl = slice(qb * 512, (qb + 1) * 512)
                po0 = aps.tile([VW, 512], F32, name=f"po0_{qb}", tag="o", bufs=2)
                po1 = aps.tile([VW, 512], F32, name=f"po1_{qb}", tag="o", bufs=2)
                for g in range(NCB // 2):
                    pre_work(qb, g)
                    cb0, cb1 = 2 * g, 2 * g + 1
                    s0 = aps.tile([128, 1024], F32, name=f"s0_{qb}_{g}",
                                  tag="s", bufs=2)
                    s1 = aps.tile([128, 1024], F32, name=f"s1_{qb}_{g}",
                                  tag="s", bufs=2)
                    for i, cb in ((0, cb0), (1, cb1)):
                        csl = slice(cb * 128, (cb + 1) * 128)
                        nc.tensor.matmul(s0[:, i * 512:(i + 1) * 512],
                                         kT_sb[0:64, csl], qT_sb[0:64, qsl],
                                         start=True, stop=True,
                                         tile_position=(0, 0) if rowtile else None)
                        nc.tensor.matmul(s1[:, i * 512:(i + 1) * 512],
                                         kT_sb[64:128, csl], qT_sb[64:128, qsl],
                                         start=True, stop=True,
                                         tile_position=(64, 0) if rowtile else None)
                    p0 = psb.tile([128, 1024], F32R, name=f"p0_{qb}_{g}", tag="p", bufs=6)
                    p1 = psb.tile([128, 1024], F32R, name=f"p1_{qb}_{g}", tag="p", bufs=6)
                    nc.scalar.activation(p0[:], s0[:], EXP, scale=SCALE)
                    nc.scalar.activation(p1[:], s1[:], EXP, scale=SCALE)
                    for i, cb in ((0, cb0), (1, cb1)):
                        vsl = slice(cb * VW, cb * VW + VW)
                        nc.tensor.matmul(po0[:], v0_sb[:, vsl],
                                         p0[:, i * 512:(i + 1) * 512],
                                         start=(g == 0 and i == 0),
                                         stop=(g == NCB // 2 - 1 and i == 1))
                        nc.tensor.matmul(po1[:], v1_sb[:, vsl],
                                         p1[:, i * 512:(i + 1) * 512],
                                         start=(g == 0 and i == 0),
                                         stop=(g == NCB // 2 - 1 and i == 1))
                # softmax normalization: oT = o_aug[0:64] / denom
                ot0 = psb.tile([64, 512], F32R, name=f"ot0_{qb}", tag="ot", bufs=4)
                ot1 = psb.tile([64, 512], F32R, name=f"ot1_{qb}", tag="ot", bufs=4)
                for hl, po, oT in ((0, po0, ot0), (1, po1, ot1)):
                    den = msb.tile([1, 512], F32, name=f"den{hl}_{qb}", tag="den")
                    nc.vector.tensor_copy(den[:], po[64:65, :])
                    rc = msb.tile([1, 512], F32, name=f"rc{hl}_{qb}", tag="rc")
                    nc.vector.reciprocal(rc[:], den[:])
                    if qb == 0 and hl == 0:
                        nc.sync.dma_start(dbg_den, den[:])
                        nc.sync.dma_start(dbg_rc, rc[:])
                    bc = aps.tile([64, 512], F32, name=f"bc{hl}_{qb}",
                                  tag="bc", bufs=1)
                    nc.tensor.matmul(bc[:], onesk_sb[:], rc[:],
                                     start=True, stop=True)
                    bcs = msb.tile([64, 512], F32, name=f"bcs{hl}_{qb}", tag="bcs")
                    nc.vector.tensor_copy(bcs[:], bc[:])
                    nc.vector.tensor_mul(oT[:], po[0:64, :], bcs[:])
                # output projection for this q-block
                for sc in range(4):
                    r0 = qb * 512 + sc * 128
                    ssl = slice(r0, r0 + 128)
                    py = aps.tile([128, D], F32, name=f"py_{qb}_{sc}",
                                  tag="py", bufs=1)
                    nc.tensor.matmul(py[:], ot0[:, sc * 128:(sc + 1) * 128],
                                     wo0_sb[:], start=True, stop=False)
                    nc.tensor.matmul(py[:], ot1[:, sc * 128:(sc + 1) * 128],
                                     wo1_sb[:], start=False, stop=True)
                    ysb = msb.tile([128, D], F32, name=f"y_{qb}_{sc}", tag="y")
                    nc.vector.tensor_copy(ysb[:], py[:])
                    nc.sync.dma_start(y[ssl, :], ysb[:])

    nc.compile()
    return nc


def make_in_maps(x, context, w_q, w_k, w_v, w_out):
    wqT = round_tf32(w_q.T)    # [D, INNER]
    wkT = round_tf32(w_k.T)
    wvT = round_tf32(w_v.T)
    woT = round_tf32(w_out.T)  # [INNER, D]
    vones = np.ones((128, NCB), dtype=np.float32)
    onesk = np.ones((1, DH), dtype=np.float32)
    xTs = [round_tf32(x[b].T) for b in range(B)]
    cTs = [round_tf32(context[b].T) for b in range(B)]
    in_maps = []
    for c in range(8):
        b, hp = c // 4, c % 4
        hsl = slice(hp * 128, (hp + 1) * 128)
        in_maps.append({
            "xT": xTs[b],
            "ctxT": cTs[b],
            "wqT": np.ascontiguousarray(wqT[:, hsl]),
            "wkT": np.ascontiguousarray(wkT[:, hsl]),
            "wvT": np.ascontiguousarray(wvT[:, hsl]),
            "woT": np.ascontiguousarray(woT[hsl, :]),
            "vones": vones,
            "onesk": onesk,
        })
    return in_maps


def kernel(x, context, w_q, w_k, w_v, w_out, b_out):
    x = np.asarray(x, dtype=np.float32)
    context = np.asarray(context, dtype=np.float32)
    w_q = np.asarray(w_q, dtype=np.float32)
    w_k = np.asarray(w_k, dtype=np.float32)
    w_v = np.asarray(w_v, dtype=np.float32)
    w_out = np.asarray(w_out, dtype=np.float32)
    b_out = np.asarray(b_out, dtype=np.float32)

    if "nc" not in _CACHE:
        _CACHE["nc"] = build_nc()
    nc = _CACHE["nc"]

    in_maps = make_in_maps(x, context, w_q, w_k, w_v, w_out)
    res = run_bass_kernel_spmd(nc, in_maps, list(range(8))).results
    _CACHE["res0"] = res[0]

    out = np.zeros((B, S, D), dtype=np.float32)
    for c in range(8):
        out[c // 4] += res[c]["y"]
    out += b_out
    return out



# revision 3
# speedup vs baseline: 1.1365x; 1.1365x over previous
"""Trainium2 Bass kernel for nn_CrossAttention (B=2, S=C=4096, D=512, H=8, Dh=64).

Sharding: batch x head-pair parallel over 8 cores. Core c handles batch
b = c//4 and heads {2*(c%4), 2*(c%4)+1}. Each core computes full attention
for its two heads plus its partial contribution to the output projection;
the host sums the 4 per-core partials per batch and adds the bias.

All-bf16 dataflow (inputs pre-rounded on host; fp32 PSUM accumulation):
  kT [128=2*dh, C] = wk_sb.T @ ctx_ch          (N=512 moving)
  qT [128=2*dh, S] = wq_sb.T @ x_ch
  v  [c, 128=2*dh] = ctx_ch.T @ wv_sb          (N=128 moving, bf16)
  sT chunk [128c, 512q] = kT_h.T @ qT_h        -> PSUM f32
  P = exp(SCALE * sT) -> bf16 SBUF             (ACT exp for most chunk
      groups; a few go to DVE as Schraudolph bit-trick exp:
      bf16_bits = int16(A*s + B), written via int16 bitcast)
  o_aug [128q, 65] += P_chunk.T @ v_aug_chunk  (P stationary, v moving
      N=65; col 64 = ones -> softmax denominator lands per-q-partition)
  o2n [128q, 128i] = o_aug * recip(den)        (DVE per-partition scalar)
  oT  [128i, 128q] = XBAR dma transpose of o2n
  y   [128s, 512]  = oT.T @ woT                (K=128, both heads at once)
"""

import os
import numpy as np
import ml_dtypes
from contextlib import ExitStack

import concourse.bass as bass
import concourse.tile as tile
from concourse import bacc, mybir
from concourse.bass_utils import run_bass_kernel_spmd

F32 = mybir.dt.float32
BF16 = mybir.dt.bfloat16
I16 = mybir.dt.int16
EXP = mybir.ActivationFunctionType.Exp
MULT = mybir.AluOpType.mult
ADD = mybir.AluOpType.add

B = 2
S = 4096
C = 4096
D = 512
DH = 64
SCALE = DH ** -0.5  # 0.125
NKC = D // 128      # 4 contraction chunks
NQB = S // 512      # 8 query blocks
NCB = C // 128      # 32 context chunks of 128
NG = NCB // 2       # 16 chunk groups of 2 per (h, qb)
VW = DH + 1         # 65

# Schraudolph exp in bf16-bits domain: bits = int16(A*s + B)
SCH_A = SCALE * 128.0 / float(np.log(2.0))
SCH_B = 16256.0 - 5.25
# which of the 16 chunk groups per (h, qb) run exp on DVE instead of ACT
DVE_GROUPS = frozenset(
    int(t) for t in os.environ.get("ATT_DVE_GROUPS", "3,6,9,12,15").split(",") if t
)

_CACHE = {}


def build_nc():
    nc = bacc.Bacc("TRN2", target_bir_lowering=False, debug=False)

    xT = nc.dram_tensor("xT", [D, S], BF16, kind="ExternalInput").ap()
    ctxT = nc.dram_tensor("ctxT", [D, C], BF16, kind="ExternalInput").ap()
    wqT = nc.dram_tensor("wqT", [D, 128], BF16, kind="ExternalInput").ap()
    wkT = nc.dram_tensor("wkT", [D, 128], BF16, kind="ExternalInput").ap()
    wvT = nc.dram_tensor("wvT", [D, 128], BF16, kind="ExternalInput").ap()
    woT = nc.dram_tensor("woT", [128, D], BF16, kind="ExternalInput").ap()
    vones = nc.dram_tensor("vones", [128, NCB], BF16, kind="ExternalInput").ap()
    y = nc.dram_tensor("y", [S, D], F32, kind="ExternalOutput").ap()

    def o_off(qs, h):
        # o_aug slice offsets inside the [128, 1024] accumulator; each
        # 65-wide slice must stay inside a 512-word PSUM bank.
        return (qs // 2) * 512 + (qs % 2) * 2 * VW + h * VW

    with tile.TileContext(nc) as tc, ExitStack() as ctx:
        sb = ctx.enter_context(tc.tile_pool(name="sb", bufs=1))

        # ---- persistent SBUF tiles ----
        wq_sb = sb.tile([128, D], BF16, name="wq_sb")
        wk_sb = sb.tile([128, D], BF16, name="wk_sb")
        wv_sb = sb.tile([128, D], BF16, name="wv_sb")
        wo_sb = sb.tile([128, D], BF16, name="wo_sb")
        kT_sb = sb.tile([128, C], BF16, name="kT_sb")
        qT_sb = sb.tile([128, S], BF16, name="qT_sb")
        v0_sb = sb.tile([128, NCB * VW], BF16, name="v0_sb")
        v1_sb = sb.tile([128, NCB * VW], BF16, name="v1_sb")

        for kc in range(NKC):
            ks = slice(kc * 128, (kc + 1) * 128)
            nc.sync.dma_start(wq_sb[:, ks], wqT[ks, :])
            nc.sync.dma_start(wk_sb[:, ks], wkT[ks, :])
            nc.sync.dma_start(wv_sb[:, ks], wvT[ks, :])
        nc.sync.dma_start(wo_sb[:], woT)
        # ones columns of v_aug (position 64 of each 65-wide chunk)
        v0_3d = v0_sb.rearrange("p (c k) -> p c k", k=VW)
        v1_3d = v1_sb.rearrange("p (c k) -> p c k", k=VW)
        nc.sync.dma_start(v0_3d[:, :, DH:VW], vones.unsqueeze(2))
        nc.sync.dma_start(v1_3d[:, :, DH:VW], vones.unsqueeze(2))

        with tc.tile_pool(name="aps", bufs=1, space="PSUM") as aps, \
             tc.tile_pool(name="inbig", bufs=16) as inbig, \
             tc.tile_pool(name="psb", bufs=6) as psb, \
             tc.tile_pool(name="msb", bufs=2) as msb:
            # input halves; attention-critical ctx first
            ctx_ch = [None] * NKC
            x_ch = [None] * NKC
            for kc in range(NKC):
                ctx_ch[kc] = inbig.tile([128, C], BF16, name=f"ctx{kc}", tag="in")
            for kc in range(NKC):
                x_ch[kc] = inbig.tile([128, S], BF16, name=f"x{kc}", tag="in")
            for h in range(2):
                hs = slice(h * 2048, (h + 1) * 2048)
                for kc in range(NKC):
                    nc.sync.dma_start(ctx_ch[kc][:, hs],
                                      ctxT[kc * 128:(kc + 1) * 128, hs])
                for kc in range(NKC):
                    nc.sync.dma_start(x_ch[kc][:, hs],
                                      xT[kc * 128:(kc + 1) * 128, hs])

            def kproj(n):
                pk = aps.tile([128, 512], F32, name=f"pk{n}", tag="y", bufs=2)
                for kc in range(NKC):
                    nc.tensor.matmul(pk[:], wk_sb[:, kc * 128:(kc + 1) * 128],
                                     ctx_ch[kc][:, n * 512:(n + 1) * 512],
                                     start=(kc == 0), stop=(kc == NKC - 1))
                nc.vector.tensor_copy(kT_sb[:, n * 512:(n + 1) * 512], pk[:])

            def qproj(qb):
                pq = aps.tile([128, 512], F32, name=f"pq{qb}", tag="y", bufs=2)
                for kc in range(NKC):
                    nc.tensor.matmul(pq[:], wq_sb[:, kc * 128:(kc + 1) * 128],
                                     x_ch[kc][:, qb * 512:(qb + 1) * 512],
                                     start=(kc == 0), stop=(kc == NKC - 1))
                nc.vector.tensor_copy(qT_sb[:, qb * 512:(qb + 1) * 512], pq[:])

            def vproj(cb):
                pv = aps.tile([128, 128], F32, name=f"pv{cb}", tag="y", bufs=2)
                for kc in range(NKC):
                    nc.tensor.matmul(pv[:],
                                     ctx_ch[kc][:, cb * 128:(cb + 1) * 128],
                                     wv_sb[:, kc * 128:(kc + 1) * 128],
                                     start=(kc == 0), stop=(kc == NKC - 1))
                nc.vector.tensor_copy(v0_sb[:, cb * VW:cb * VW + DH], pv[:, 0:64])
                nc.vector.tensor_copy(v1_sb[:, cb * VW:cb * VW + DH], pv[:, 64:128])

            for n in range(NQB):
                kproj(n)
            qproj(0)
            vproj(0)
            vproj(1)

            def pre_work(qb, h, g):
                if qb == 0 and h == 0 and g < NG - 1:
                    vproj(2 * g + 2)
                    vproj(2 * g + 3)
                if h == 1 and g == 0 and qb + 1 < NQB:
                    qproj(qb + 1)

            # ---- attention main loop ----
            for qb in range(NQB):
                qsl = slice(qb * 512, (qb + 1) * 512)
                o_ps = aps.tile([128, 1024], F32, name=f"o{qb}", tag="o", bufs=1)
                pend = None  # (p_sb, h, g) whose PV is not yet emitted

                def flush_pv(p_sb, h, g):
                    # start=True clears the ENTIRE psum bank's has_written
                    # bits, so only the first matmul into each bank (qs 0 and
                    # 2, h==0, g==0, i==0) may carry it; every other region's
                    # first write then overwrites via per-element has_written.
                    v_sb = v0_sb if h == 0 else v1_sb
                    for i, cb in ((0, 2 * g), (1, 2 * g + 1)):
                        vsl = slice(cb * VW, (cb + 1) * VW)
                        for qs in range(4):
                            off = o_off(qs, h)
                            nc.tensor.matmul(
                                o_ps[:, off:off + VW],
                                p_sb[:, i * 512 + qs * 128:
                                     i * 512 + (qs + 1) * 128],
                                v_sb[:, vsl],
                                start=(h == 0 and g == 0 and i == 0
                                       and qs % 2 == 0),
                                stop=(h == 1 and g == NG - 1 and i == 1),
                                skip_group_check=True)

                for h in range(2):
                    hsl = slice(h * 64, (h + 1) * 64)
                    for g in range(NG):
                        s_ps = aps.tile([128, 1024], F32, name=f"s{qb}_{h}_{g}",
                                        tag="s", bufs=2)
                        for i, cb in ((0, 2 * g), (1, 2 * g + 1)):
                            csl = slice(cb * 128, (cb + 1) * 128)
                            nc.tensor.matmul(s_ps[:, i * 512:(i + 1) * 512],
                                             kT_sb[hsl, csl], qT_sb[hsl, qsl],
                                             start=True, stop=True)
                        p_sb = psb.tile([128, 1024], BF16,
                                        name=f"p{qb}_{h}_{g}", tag="p", bufs=6)
                        if g in DVE_GROUPS:
                            nc.vector.tensor_scalar(
                                p_sb[:].bitcast(I16), s_ps[:],
                                SCH_A, SCH_B, MULT, ADD)
                        else:
                            nc.scalar.activation(p_sb[:], s_ps[:], EXP,
                                                 scale=SCALE)
                        if pend is not None:
                            flush_pv(*pend)
                        pend = (p_sb, h, g)
                        pre_work(qb, h, g)
                flush_pv(*pend)
                pend = None

                # normalize both heads into [q, 128i] and recip the denoms
                o2n = [None] * 4
                for qs in range(4):
                    t = msb.tile([128, 128], BF16, name=f"o2n{qb}_{qs}",
                                 tag="o2n", bufs=4)
                    for h in range(2):
                        off = o_off(qs, h)
                        rc = msb.tile([128, 1], F32, name=f"rc{qb}_{qs}_{h}",
                                      tag="rc", bufs=8)
                        nc.vector.reciprocal(rc[:], o_ps[:, off + DH:off + VW])
                        nc.vector.tensor_scalar(t[:, h * 64:(h + 1) * 64],
                                                o_ps[:, off:off + DH],
                                                rc[:], None, MULT)
                    o2n[qs] = t
                oT = [None] * 4
                for qs in range(4):
                    t = msb.tile([128, 128], BF16, name=f"oT{qb}_{qs}",
                                 tag="oT", bufs=4)
                    nc.sync.dma_start_transpose(t[:], o2n[qs][:])
                    oT[qs] = t
                for qs in range(4):
                    py = aps.tile([128, 512], F32, name=f"py{qb}_{qs}",
                                  tag="y", bufs=2)
                    nc.tensor.matmul(py[:], oT[qs][:], wo_sb[:],
                                     start=True, stop=True)
                    ysb = msb.tile([128, 512], F32, name=f"y{qb}_{qs}",
                                   tag="ysb", bufs=2)
                    nc.vector.tensor_copy(ysb[:], py[:])
                    r0 = qb * 512 + qs * 128
                    nc.sync.dma_start(y[r0:r0 + 128, :], ysb[:])

    nc.compile()
    return nc


def make_in_maps(x, context, w_q, w_k, w_v, w_out):
    bf = ml_dtypes.bfloat16
    wqT = np.ascontiguousarray(w_q.T).astype(bf)    # [D, INNER]
    wkT = np.ascontiguousarray(w_k.T).astype(bf)
    wvT = np.ascontiguousarray(w_v.T).astype(bf)
    woT = np.ascontiguousarray(w_out.T).astype(bf)  # [INNER, D]
    vones = np.ones((128, NCB), dtype=bf)
    xTs = [np.ascontiguousarray(x[b].T).astype(bf) for b in range(B)]
    cTs = [np.ascontiguousarray(context[b].T).astype(bf) for b in range(B)]
    in_maps = []
    for c in range(8):
        b, hp = c // 4, c % 4
        hsl = slice(hp * 128, (hp + 1) * 128)
        in_maps.append({
            "xT": xTs[b],
            "ctxT": cTs[b],
            "wqT": np.ascontiguousarray(wqT[:, hsl]),
            "wkT": np.ascontiguousarray(wkT[:, hsl]),
            "wvT": np.ascontiguousarray(wvT[:, hsl]),
            "woT": np.ascontiguousarray(woT[hsl, :]),
            "vones": vones,
        })
    return in_maps


def kernel(x, context, w_q, w_k, w_v, w_out, b_out):
    x = np.asarray(x, dtype=np.float32)
    context = np.asarray(context, dtype=np.float32)
    w_q = np.asarray(w_q, dtype=np.float32)
    w_k = np.asarray(w_k, dtype=np.float32)
    w_v = np.asarray(w_v, dtype=np.float32)
    w_out = np.asarray(w_out, dtype=np.float32)
    b_out = np.asarray(b_out, dtype=np.float32)

    if "nc" not in _CACHE:
        _CACHE["nc"] = build_nc()
    nc = _CACHE["nc"]

    in_maps = make_in_maps(x, context, w_q, w_k, w_v, w_out)
    res = run_bass_kernel_spmd(nc, in_maps, list(range(8))).results

    out = np.zeros((B, S, D), dtype=np.float32)
    for c in range(8):
        out[c // 4] += np.asarray(res[c]["y"], dtype=np.float32)
    out += b_out
    return out


# revision 4
# speedup vs baseline: 1.1827x; 1.0406x over previous
"""Trainium2 Bass kernel for nn_CrossAttention (B=2, S=C=4096, D=512, H=8, Dh=64).

Sharding: batch x head-pair parallel over 8 cores. Core c handles batch
b = c//4 and heads {2*(c%4), 2*(c%4)+1}. Each core computes full attention
for its two heads plus its partial contribution to the output projection;
the host sums the 4 per-core partials per batch and adds the bias.

All-bf16 dataflow (inputs pre-rounded on host; fp32 PSUM accumulation):
  kT [128=2*dh, C] = wk_sb.T @ ctx_ch          (N=512 moving)
  qT [128=2*dh, S] = wq_sb.T @ x_ch
  v  [c, 128=2*dh] = ctx_ch.T @ wv_sb          (N=128 moving, bf16)
  sT chunk [128c, 512q] = kT_h.T @ qT_h        -> PSUM f32
  P = exp(SCALE * sT) -> bf16 SBUF             (ACT exp for most chunk
      groups; some go to DVE as Schraudolph bit-trick exp:
      bf16_bits = int16(A*s + B), written via int16 bitcast)
  o_aug [128q, 65] += P_chunk.T @ v_aug_chunk  (P stationary, v moving
      N=65; col 64 = ones -> softmax denominator lands per-q-partition)
  o2n [128q, 128i] = o_aug * recip(den)        (DVE per-partition scalar)
  oT  [128i, 128q] = XBAR dma transpose of o2n
  y   [128s, 512]  = oT.T @ woT                (K=128, both heads at once)

The per-qb epilogue (normalize / transpose / out-proj / y copy) is
software-pipelined into the NEXT qb's h=0 chunk groups so no engine sits
behind the long-latency transpose chain at block boundaries.
"""

import os
import numpy as np
import ml_dtypes
from contextlib import ExitStack

import concourse.bass as bass
import concourse.tile as tile
from concourse import bacc, mybir
from concourse.bass_utils import run_bass_kernel_spmd

F32 = mybir.dt.float32
BF16 = mybir.dt.bfloat16
I16 = mybir.dt.int16
EXP = mybir.ActivationFunctionType.Exp
MULT = mybir.AluOpType.mult
ADD = mybir.AluOpType.add

B = 2
S = 4096
C = 4096
D = 512
DH = 64
SCALE = DH ** -0.5  # 0.125
NKC = D // 128      # 4 contraction chunks
NQB = S // 512      # 8 query blocks
NCB = C // 128      # 32 context chunks of 128
NG = NCB // 2       # 16 chunk groups of 2 per (h, qb)
VW = DH + 1         # 65
VROW = NCB * VW     # per-head width of the v_aug tile

# Schraudolph exp in bf16-bits domain: bits = int16(A*s + B)
SCH_A = SCALE * 128.0 / float(np.log(2.0))
SCH_B = 16256.0 - 5.25
# which chunk groups run exp on DVE instead of ACT, per h
DVE_H = (
    frozenset(int(t) for t in
              os.environ.get("ATT_DVE_H0", "3,6,9,12,14").split(",") if t),
    frozenset(int(t) for t in
              os.environ.get("ATT_DVE_H1", "1,4,7,10,13").split(",") if t),
)

_CACHE = {}


def build_nc():
    nc = bacc.Bacc("TRN2", target_bir_lowering=False, debug=False)

    xT = nc.dram_tensor("xT", [D, S], BF16, kind="ExternalInput").ap()
    ctxT = nc.dram_tensor("ctxT", [D, C], BF16, kind="ExternalInput").ap()
    wqT = nc.dram_tensor("wqT", [D, 128], BF16, kind="ExternalInput").ap()
    wkT = nc.dram_tensor("wkT", [D, 128], BF16, kind="ExternalInput").ap()
    wvT = nc.dram_tensor("wvT", [D, 128], BF16, kind="ExternalInput").ap()
    woT = nc.dram_tensor("woT", [128, D], BF16, kind="ExternalInput").ap()
    vones = nc.dram_tensor("vones", [128, 2 * NCB], BF16,
                           kind="ExternalInput").ap()
    y = nc.dram_tensor("y", [S, D], F32, kind="ExternalOutput").ap()

    def o_off(qs, h):
        # o_aug slice offsets inside the [128, 1024] accumulator; each
        # 65-wide slice must stay inside a 512-word PSUM bank.
        return (qs // 2) * 512 + (qs % 2) * 2 * VW + h * VW

    with tile.TileContext(nc) as tc, ExitStack() as ctx:
        sb = ctx.enter_context(tc.tile_pool(name="sb", bufs=1))

        # ---- persistent SBUF tiles ----
        wq_sb = sb.tile([128, D], BF16, name="wq_sb")
        wk_sb = sb.tile([128, D], BF16, name="wk_sb")
        wv_sb = sb.tile([128, D], BF16, name="wv_sb")
        wo_sb = sb.tile([128, D], BF16, name="wo_sb")
        kT_sb = sb.tile([128, C], BF16, name="kT_sb")
        qT_sb = sb.tile([128, S], BF16, name="qT_sb")
        v_sb = sb.tile([128, 2 * VROW], BF16, name="v_sb")

        with tc.tile_pool(name="aps", bufs=1, space="PSUM") as aps, \
             tc.tile_pool(name="inbig", bufs=16) as inbig, \
             tc.tile_pool(name="psb", bufs=6) as psb, \
             tc.tile_pool(name="msb", bufs=2) as msb:
            ctx_ch = [inbig.tile([128, C], BF16, name=f"ctx{kc}", tag="in")
                      for kc in range(NKC)]
            x_ch = [inbig.tile([128, S], BF16, name=f"x{kc}", tag="in")
                    for kc in range(NKC)]

            # ---- DMA order tuned for ramp: k/q weights, first input
            # quarters (enough for kproj(0..1)/qproj(0)/vproj(0..1)),
            # then everything else.
            def dma_w(dst, src):
                nc.sync.dma_start(
                    dst.rearrange("p (kc m) -> p kc m", m=128),
                    src.rearrange("(kc p) m -> p kc m", p=128))

            dma_w(wk_sb, wkT)
            dma_w(wq_sb, wqT)
            dma_w(wv_sb, wvT)
            for kc in range(NKC):
                nc.sync.dma_start(ctx_ch[kc][:, 0:1024],
                                  ctxT[kc * 128:(kc + 1) * 128, 0:1024])
            for kc in range(NKC):
                nc.sync.dma_start(x_ch[kc][:, 0:1024],
                                  xT[kc * 128:(kc + 1) * 128, 0:1024])
            nc.sync.dma_start(wo_sb[:], woT)
            v4 = v_sb.rearrange("p (h c k) -> p h c k", h=2, k=VW)
            nc.sync.dma_start(
                v4[:, :, :, DH:VW],
                vones.rearrange("p (h c) -> p h c", h=2).unsqueeze(3))
            for kc in range(NKC):
                nc.sync.dma_start(ctx_ch[kc][:, 1024:C],
                                  ctxT[kc * 128:(kc + 1) * 128, 1024:C])
            for kc in range(NKC):
                nc.sync.dma_start(x_ch[kc][:, 1024:S],
                                  xT[kc * 128:(kc + 1) * 128, 1024:S])

            def kproj(n):
                pk = aps.tile([128, 512], F32, name=f"pk{n}", tag="y", bufs=2)
                for kc in range(NKC):
                    nc.tensor.matmul(pk[:], wk_sb[:, kc * 128:(kc + 1) * 128],
                                     ctx_ch[kc][:, n * 512:(n + 1) * 512],
                                     start=(kc == 0), stop=(kc == NKC - 1))
                nc.vector.tensor_copy(kT_sb[:, n * 512:(n + 1) * 512], pk[:])

            def qproj(qb):
                pq = aps.tile([128, 512], F32, name=f"pq{qb}", tag="y", bufs=2)
                for kc in range(NKC):
                    nc.tensor.matmul(pq[:], wq_sb[:, kc * 128:(kc + 1) * 128],
                                     x_ch[kc][:, qb * 512:(qb + 1) * 512],
                                     start=(kc == 0), stop=(kc == NKC - 1))
                nc.vector.tensor_copy(qT_sb[:, qb * 512:(qb + 1) * 512], pq[:])

            def vproj(cb):
                pv = aps.tile([128, 128], F32, name=f"pv{cb}", tag="y", bufs=2)
                for kc in range(NKC):
                    nc.tensor.matmul(pv[:],
                                     ctx_ch[kc][:, cb * 128:(cb + 1) * 128],
                                     wv_sb[:, kc * 128:(kc + 1) * 128],
                                     start=(kc == 0), stop=(kc == NKC - 1))
                nc.vector.tensor_copy(
                    v4[:, :, cb, 0:DH],
                    pv.rearrange("p (h m) -> p h m", m=DH))

            kproj(0)
            qproj(0)
            vproj(0)
            vproj(1)
            for n in range(1, NQB):
                kproj(n)

            def pre_work(qb, h, g):
                if qb == 0 and h == 0 and g < NG - 1:
                    vproj(2 * g + 2)
                    vproj(2 * g + 3)
                if h == 1 and g == 0 and qb + 1 < NQB:
                    qproj(qb + 1)

            # ---- epilogue pieces (for the PREVIOUS qb) ----
            def ep_norm(o_ps_p, qbp):
                oTs = []
                for qs in range(4):
                    t = msb.tile([128, 128], BF16, name=f"o2n{qbp}_{qs}",
                                 tag="o2n", bufs=4)
                    for h in range(2):
                        off = o_off(qs, h)
                        rc = msb.tile([128, 1], F32, name=f"rc{qbp}_{qs}_{h}",
                                      tag="rc", bufs=8)
                        nc.vector.reciprocal(rc[:],
                                             o_ps_p[:, off + DH:off + VW])
                        nc.vector.tensor_scalar(t[:, h * 64:(h + 1) * 64],
                                                o_ps_p[:, off:off + DH],
                                                rc[:], None, MULT)
                    ot = msb.tile([128, 128], BF16, name=f"oT{qbp}_{qs}",
                                  tag="oT", bufs=4)
                    nc.sync.dma_start_transpose(ot[:], t[:])
                    oTs.append(ot)
                return oTs

            def ep_py(qbp, oTs, qs):
                py = aps.tile([128, 512], F32, name=f"py{qbp}_{qs}",
                              tag="y", bufs=2)
                nc.tensor.matmul(py[:], oTs[qs][:], wo_sb[:],
                                 start=True, stop=True)
                return py

            def ep_ycopy(qbp, pys, qs):
                ysb = msb.tile([128, 512], F32, name=f"y{qbp}_{qs}",
                               tag="ysb", bufs=2)
                nc.vector.tensor_copy(ysb[:], pys[qs][:])
                r0 = qbp * 512 + qs * 128
                nc.sync.dma_start(y[r0:r0 + 128, :], ysb[:])

            # ---- attention main loop ----
            prev_ep = None  # (o_ps, qb) awaiting epilogue
            for qb in range(NQB):
                qsl = slice(qb * 512, (qb + 1) * 512)
                ep_oTs = None
                ep_pys = {}
                if prev_ep is not None:
                    ep_oTs = ep_norm(*prev_ep)
                o_ps = aps.tile([128, 1024], F32, name=f"o{qb}", tag="o",
                                bufs=1)
                pend = None

                def flush_pv(p_sb, h, g):
                    # start=True clears the ENTIRE psum bank's has_written
                    # bits, so only the first matmul into each bank (qs 0/2,
                    # h==0, g==0, i==0) may carry it; every other region's
                    # first write then overwrites via per-element has_written.
                    for i, cb in ((0, 2 * g), (1, 2 * g + 1)):
                        vsl = slice(h * VROW + cb * VW,
                                    h * VROW + (cb + 1) * VW)
                        for qs in range(4):
                            off = o_off(qs, h)
                            nc.tensor.matmul(
                                o_ps[:, off:off + VW],
                                p_sb[:, i * 512 + qs * 128:
                                     i * 512 + (qs + 1) * 128],
                                v_sb[:, vsl],
                                start=(h == 0 and g == 0 and i == 0
                                       and qs % 2 == 0),
                                stop=(h == 1 and g == NG - 1 and i == 1),
                                skip_group_check=True)

                for h in range(2):
                    hsl = slice(h * 64, (h + 1) * 64)
                    for g in range(NG):
                        s_ps = aps.tile([128, 1024], F32,
                                        name=f"s{qb}_{h}_{g}", tag="s", bufs=2)
                        for i, cb in ((0, 2 * g), (1, 2 * g + 1)):
                            csl = slice(cb * 128, (cb + 1) * 128)
                            nc.tensor.matmul(s_ps[:, i * 512:(i + 1) * 512],
                                             kT_sb[hsl, csl], qT_sb[hsl, qsl],
                                             start=True, stop=True)
                        p_sb = psb.tile([128, 1024], BF16,
                                        name=f"p{qb}_{h}_{g}", tag="p", bufs=6)
                        if g in DVE_H[h]:
                            nc.vector.tensor_scalar(
                                p_sb[:].bitcast(I16), s_ps[:],
                                SCH_A, SCH_B, MULT, ADD)
                        else:
                            nc.scalar.activation(p_sb[:], s_ps[:], EXP,
                                                 scale=SCALE)
                        if pend is not None:
                            flush_pv(*pend)
                        pend = (p_sb, h, g)
                        # pipelined epilogue of the previous qb
                        if h == 0 and prev_ep is not None:
                            if 4 <= g < 8:
                                ep_pys[g - 4] = ep_py(prev_ep[1], ep_oTs,
                                                      g - 4)
                            if 5 <= g < 9:
                                ep_ycopy(prev_ep[1], ep_pys, g - 5)
                        pre_work(qb, h, g)
                flush_pv(*pend)
                pend = None
                prev_ep = (o_ps, qb)

            # final epilogue (qb = NQB-1)
            oTs = ep_norm(*prev_ep)
            pys = {}
            for qs in range(4):
                pys[qs] = ep_py(prev_ep[1], oTs, qs)
            for qs in range(4):
                ep_ycopy(prev_ep[1], pys, qs)

    nc.compile()
    return nc


def make_in_maps(x, context, w_q, w_k, w_v, w_out):
    bf = ml_dtypes.bfloat16
    wqT = np.ascontiguousarray(w_q.T).astype(bf)    # [D, INNER]
    wkT = np.ascontiguousarray(w_k.T).astype(bf)
    wvT = np.ascontiguousarray(w_v.T).astype(bf)
    woT = np.ascontiguousarray(w_out.T).astype(bf)  # [INNER, D]
    vones = np.ones((128, 2 * NCB), dtype=bf)
    xTs = [np.ascontiguousarray(x[b].T).astype(bf) for b in range(B)]
    cTs = [np.ascontiguousarray(context[b].T).astype(bf) for b in range(B)]
    in_maps = []
    for c in range(8):
        b, hp = c // 4, c % 4
        hsl = slice(hp * 128, (hp + 1) * 128)
        in_maps.append({
            "xT": xTs[b],
            "ctxT": cTs[b],
            "wqT": np.ascontiguousarray(wqT[:, hsl]),
            "wkT": np.ascontiguousarray(wkT[:, hsl]),
            "wvT": np.ascontiguousarray(wvT[:, hsl]),
            "woT": np.ascontiguousarray(woT[hsl, :]),
            "vones": vones,
        })
    return in_maps


def kernel(x, context, w_q, w_k, w_v, w_out, b_out):
    x = np.asarray(x, dtype=np.float32)
    context = np.asarray(context, dtype=np.float32)
    w_q = np.asarray(w_q, dtype=np.float32)
    w_k = np.asarray(w_k, dtype=np.float32)
    w_v = np.asarray(w_v, dtype=np.float32)
    w_out = np.asarray(w_out, dtype=np.float32)
    b_out = np.asarray(b_out, dtype=np.float32)

    if "nc" not in _CACHE:
        _CACHE["nc"] = build_nc()
    nc = _CACHE["nc"]

    in_maps = make_in_maps(x, context, w_q, w_k, w_v, w_out)
    res = run_bass_kernel_spmd(nc, in_maps, list(range(8))).results

    out = np.zeros((B, S, D), dtype=np.float32)
    for c in range(8):
        out[c // 4] += np.asarray(res[c]["y"], dtype=np.float32)
    out += b_out
    return out


# revision 11
# speedup vs baseline: 1.2667x; 1.0710x over previous
"""Trainium2 Bass kernel for nn_CrossAttention (B=2, S=C=4096, D=512, H=8, Dh=64).

Sharding: batch x head-pair parallel over 8 cores. Core c handles batch
b = c//4 and heads {2*(c%4), 2*(c%4)+1}. Each core computes full attention
for its two heads plus its partial contribution to the output projection;
the host sums the 4 per-core partials per batch and adds the bias.

All-bf16 dataflow (inputs pre-rounded on host; fp32 PSUM accumulation):
  kT [128=2*dh, C] = wk_sb.T @ ctx_ch          (N=512 moving)
  qT [128=2*dh, S] = wq_sb.T @ x_ch
  v  [c, 128=2*dh] = ctx_ch.T @ wv_sb          (N=128 moving, bf16)
  sT chunk [128c, 512q] = kT_h.T @ qT_h        -> PSUM f32
  P = exp(SCALE * sT) -> bf16 SBUF             (ACT exp for most chunk
      groups; some go to DVE as Schraudolph bit-trick exp:
      bf16_bits = int16(A*s + B), written via int16 bitcast)
  o_aug [128q, 65] += P_chunk.T @ v_aug_chunk  (P stationary, v moving
      N=65; col 64 = ones -> softmax denominator lands per-q-partition)
  o2n [128q, 128i] = o_aug * recip(den)        (DVE per-partition scalar)
  oT  [128i, 128q] = XBAR dma transpose of o2n
  y   [128s, 512]  = oT.T @ woT                (K=128, both heads at once)

The per-qb epilogue (normalize / transpose / out-proj / y copy) is
software-pipelined into the NEXT qb's h=0 chunk groups so no engine sits
behind the long-latency transpose chain at block boundaries.
"""

import os
import numpy as np
import ml_dtypes
from contextlib import ExitStack

import concourse.bass as bass
import concourse.tile as tile
from concourse import bacc, mybir
from concourse.bass_utils import run_bass_kernel_spmd

F32 = mybir.dt.float32
BF16 = mybir.dt.bfloat16
I16 = mybir.dt.int16
EXP = mybir.ActivationFunctionType.Exp
MULT = mybir.AluOpType.mult
ADD = mybir.AluOpType.add

B = 2
S = 4096
C = 4096
D = 512
DH = 64
SCALE = DH ** -0.5  # 0.125
NKC = D // 128      # 4 contraction chunks
NQB = S // 512      # 8 query blocks
NCB = C // 128      # 32 context chunks of 128
NG = NCB // 2       # 16 chunk groups of 2 per (h, qb)
VW = DH + 1         # 65
VROW = NCB * VW     # per-head width of the v_aug tile

# Schraudolph exp in bf16-bits domain: bits = int16(A*s + B)
SCH_A = SCALE * 128.0 / float(np.log(2.0))
SCH_B = 16256.0 - 5.25


def dve_groups(qb, h):
    # chunk groups whose exp runs on DVE instead of ACT; qb0/h0 is kept
    # light because the v-projection copies also ride the DVE there.
    if qb == 0 and h == 0:
        return frozenset((10, 12, 14))
    if h == 0:
        return frozenset((2, 4, 6, 8, 10, 12))
    return frozenset((1, 3, 5, 7, 9, 11, 13))

_CACHE = {}


def build_nc():
    nc = bacc.Bacc("TRN2", target_bir_lowering=False, debug=False)

    xT = nc.dram_tensor("xT", [D, S], BF16, kind="ExternalInput").ap()
    ctxT = nc.dram_tensor("ctxT", [D, C], BF16, kind="ExternalInput").ap()
    wqT = nc.dram_tensor("wqT", [D, 128], BF16, kind="ExternalInput").ap()
    wkT = nc.dram_tensor("wkT", [D, 128], BF16, kind="ExternalInput").ap()
    wvT = nc.dram_tensor("wvT", [D, 128], BF16, kind="ExternalInput").ap()
    woT = nc.dram_tensor("woT", [128, D], BF16, kind="ExternalInput").ap()
    vones = nc.dram_tensor("vones", [128, 2 * NCB], BF16,
                           kind="ExternalInput").ap()
    y = nc.dram_tensor("y", [S, D], F32, kind="ExternalOutput").ap()

    def o_off(qs, h):
        # o_aug slice offsets inside the [128, 1024] accumulator; regions
        # are padded to a uniform 128-word stride (so the 8 denominators at
        # +64 form one strided AP) and stay inside their 512-word bank.
        return (qs // 2) * 512 + ((qs % 2) * 2 + h) * 128

    with tile.TileContext(nc) as tc, ExitStack() as ctx:
        sb = ctx.enter_context(tc.tile_pool(name="sb", bufs=1))

        # ---- persistent SBUF tiles ----
        wq_sb = sb.tile([128, D], BF16, name="wq_sb")
        wk_sb = sb.tile([128, D], BF16, name="wk_sb")
        wv_sb = sb.tile([128, D], BF16, name="wv_sb")
        wo_sb = sb.tile([128, D], BF16, name="wo_sb")
        kT_sb = sb.tile([128, C], BF16, name="kT_sb")
        qT_sb = sb.tile([128, S], BF16, name="qT_sb")
        v_sb = sb.tile([128, 2 * VROW], BF16, name="v_sb")

        with tc.tile_pool(name="aps", bufs=1, space="PSUM") as aps, \
             tc.tile_pool(name="inbig", bufs=16) as inbig, \
             tc.tile_pool(name="psb", bufs=6) as psb, \
             tc.tile_pool(name="msb", bufs=2) as msb:
            ctx_ch = [inbig.tile([128, C], BF16, name=f"ctx{kc}", tag="in")
                      for kc in range(NKC)]
            x_ch = [inbig.tile([128, S], BF16, name=f"x{kc}", tag="in")
                    for kc in range(NKC)]

            # ---- DMA order tuned for ramp: k/q weights, first input
            # quarters (enough for kproj(0..1)/qproj(0)/vproj(0..1)),
            # then everything else.
            def dma_w(dst, src):
                nc.sync.dma_start(
                    dst.rearrange("p (kc m) -> p kc m", m=128),
                    src.rearrange("(kc p) m -> p kc m", p=128))

            dma_w(wk_sb, wkT)
            dma_w(wq_sb, wqT)
            for kc in range(NKC):
                nc.sync.dma_start(ctx_ch[kc][:, 0:512],
                                  ctxT[kc * 128:(kc + 1) * 128, 0:512])
            for kc in range(NKC):
                nc.sync.dma_start(x_ch[kc][:, 0:512],
                                  xT[kc * 128:(kc + 1) * 128, 0:512])
            dma_w(wv_sb, wvT)
            nc.sync.dma_start(wo_sb[:], woT)
            v4 = v_sb.rearrange("p (h c k) -> p h c k", h=2, k=VW)
            nc.sync.dma_start(
                v4[:, :, :, DH:VW],
                vones.rearrange("p (h c) -> p h c", h=2).unsqueeze(3))
            for kc in range(NKC):
                nc.sync.dma_start(ctx_ch[kc][:, 512:C],
                                  ctxT[kc * 128:(kc + 1) * 128, 512:C])
            for kc in range(NKC):
                nc.sync.dma_start(x_ch[kc][:, 512:S],
                                  xT[kc * 128:(kc + 1) * 128, 512:S])

            def kproj(n):
                pk = aps.tile([128, 512], F32, name=f"pk{n}", tag="y", bufs=2)
                for kc in range(NKC):
                    nc.tensor.matmul(pk[:], wk_sb[:, kc * 128:(kc + 1) * 128],
                                     ctx_ch[kc][:, n * 512:(n + 1) * 512],
                                     start=(kc == 0), stop=(kc == NKC - 1))
                nc.vector.tensor_copy(kT_sb[:, n * 512:(n + 1) * 512], pk[:])

            def qproj(qb):
                pq = aps.tile([128, 512], F32, name=f"pq{qb}", tag="y", bufs=2)
                for kc in range(NKC):
                    nc.tensor.matmul(pq[:], wq_sb[:, kc * 128:(kc + 1) * 128],
                                     x_ch[kc][:, qb * 512:(qb + 1) * 512],
                                     start=(kc == 0), stop=(kc == NKC - 1))
                nc.vector.tensor_copy(qT_sb[:, qb * 512:(qb + 1) * 512], pq[:])

            def vproj(cb):
                pv = aps.tile([128, 128], F32, name=f"pv{cb}", tag="y", bufs=2)
                for kc in range(NKC):
                    nc.tensor.matmul(pv[:],
                                     ctx_ch[kc][:, cb * 128:(cb + 1) * 128],
                                     wv_sb[:, kc * 128:(kc + 1) * 128],
                                     start=(kc == 0), stop=(kc == NKC - 1))
                nc.vector.tensor_copy(
                    v4[:, :, cb, 0:DH],
                    pv.rearrange("p (h m) -> p h m", m=DH))

            kproj(0)
            qproj(0)
            vproj(0)
            vproj(1)
            for n in range(1, NQB):
                kproj(n)

            def pre_work(qb, h, g):
                if qb == 0 and h == 0 and g < NG - 1:
                    vproj(2 * g + 2)
                    vproj(2 * g + 3)
                if h == 1 and g == 0 and qb + 1 < NQB:
                    qproj(qb + 1)

            # ---- epilogue pieces (for the PREVIOUS qb) ----
            def ep_norm(o_ps_p, qbp):
                # one strided reciprocal over all 8 denominators (at +64 of
                # each 128-padded region), then per-region scalar multiplies
                rc = msb.tile([128, 8], F32, name=f"rc{qbp}", tag="rc",
                              bufs=2)
                dens = o_ps_p.rearrange("p (r w) -> p r w",
                                        w=128)[:, :, DH:DH + 1]
                nc.vector.reciprocal(rc[:].unsqueeze(2), dens)
                oTs = []
                for qs in range(4):
                    t = msb.tile([128, 128], BF16, name=f"o2n{qbp}_{qs}",
                                 tag="o2n", bufs=4)
                    for h in range(2):
                        off = o_off(qs, h)
                        r = (qs % 2) * 2 + h + (qs // 2) * 4
                        nc.vector.tensor_scalar(t[:, h * 64:(h + 1) * 64],
                                                o_ps_p[:, off:off + DH],
                                                rc[:, r:r + 1], None, MULT)
                    ot = msb.tile([128, 128], BF16, name=f"oT{qbp}_{qs}",
                                  tag="oT", bufs=4)
                    nc.sync.dma_start_transpose(ot[:], t[:])
                    oTs.append(ot)
                return oTs

            def ep_py(qbp, oTs, qs):
                py = aps.tile([128, 512], F32, name=f"py{qbp}_{qs}",
                              tag="y", bufs=2)
                nc.tensor.matmul(py[:], oTs[qs][:], wo_sb[:],
                                 start=True, stop=True)
                return py

            def ep_ycopy(qbp, pys, qs):
                ysb = msb.tile([128, 512], F32, name=f"y{qbp}_{qs}",
                               tag="ysb", bufs=2)
                nc.vector.tensor_copy(ysb[:], pys[qs][:])
                r0 = qbp * 512 + qs * 128
                nc.sync.dma_start(y[r0:r0 + 128, :], ysb[:])

            # ---- attention main loop ----
            prev_ep = None  # (o_ps, qb) awaiting epilogue
            for qb in range(NQB):
                qsl = slice(qb * 512, (qb + 1) * 512)
                ep_oTs = None
                ep_pys = {}
                if prev_ep is not None:
                    ep_oTs = ep_norm(*prev_ep)
                o_ps = aps.tile([128, 1024], F32, name=f"o{qb}", tag="o",
                                bufs=1)
                pend = None

                def flush_pv(p_sb, h, g):
                    # start=True clears the ENTIRE psum bank's has_written
                    # bits, so only the first matmul into each bank (qs 0/2,
                    # h==0, g==0, i==0) may carry it; every other region's
                    # first write then overwrites via per-element has_written.
                    for i, cb in ((0, 2 * g), (1, 2 * g + 1)):
                        vsl = slice(h * VROW + cb * VW,
                                    h * VROW + (cb + 1) * VW)
                        for qs in range(4):
                            off = o_off(qs, h)
                            nc.tensor.matmul(
                                o_ps[:, off:off + VW],
                                p_sb[:, i * 512 + qs * 128:
                                     i * 512 + (qs + 1) * 128],
                                v_sb[:, vsl],
                                start=(h == 0 and g == 0 and i == 0
                                       and qs % 2 == 0),
                                stop=(h == 1 and g == NG - 1 and i == 1),
                                skip_group_check=True)

                pend = []
                for h in range(2):
                    hsl = slice(h * 64, (h + 1) * 64)
                    dve_g = dve_groups(qb, h)
                    for g in range(NG):
                        s_ps = aps.tile([128, 1024], F32,
                                        name=f"s{qb}_{h}_{g}", tag="s", bufs=2)
                        for i, cb in ((0, 2 * g), (1, 2 * g + 1)):
                            csl = slice(cb * 128, (cb + 1) * 128)
                            nc.tensor.matmul(s_ps[:, i * 512:(i + 1) * 512],
                                             kT_sb[hsl, csl], qT_sb[hsl, qsl],
                                             start=True, stop=True)
                        p_sb = psb.tile([128, 1024], BF16,
                                        name=f"p{qb}_{h}_{g}", tag="p", bufs=6)
                        if g in dve_g:
                            nc.vector.tensor_scalar(
                                p_sb[:].bitcast(I16), s_ps[:],
                                SCH_A, SCH_B, MULT, ADD)
                        else:
                            nc.scalar.activation(p_sb[:], s_ps[:], EXP,
                                                 scale=SCALE)
                        # PV lags exp by 2 groups so the PE never parks
                        # behind the o_ps WAR (previous qb's normalize)
                        pend.append((p_sb, h, g))
                        if len(pend) > 2:
                            flush_pv(*pend.pop(0))
                        # pipelined epilogue of the previous qb
                        if h == 0 and prev_ep is not None:
                            if 4 <= g < 8:
                                ep_pys[g - 4] = ep_py(prev_ep[1], ep_oTs,
                                                      g - 4)
                            if 6 <= g < 10:
                                ep_ycopy(prev_ep[1], ep_pys, g - 6)
                        pre_work(qb, h, g)
                for t in pend:
                    flush_pv(*t)
                pend = []
                prev_ep = (o_ps, qb)

            # final epilogue (qb = NQB-1)
            oTs = ep_norm(*prev_ep)
            pys = {}
            for qs in range(4):
                pys[qs] = ep_py(prev_ep[1], oTs, qs)
            for qs in range(4):
                ep_ycopy(prev_ep[1], pys, qs)

    nc.compile()
    return nc


def make_in_maps(x, context, w_q, w_k, w_v, w_out):
    bf = ml_dtypes.bfloat16
    wqT = np.ascontiguousarray(w_q.T).astype(bf)    # [D, INNER]
    wkT = np.ascontiguousarray(w_k.T).astype(bf)
    wvT = np.ascontiguousarray(w_v.T).astype(bf)
    woT = np.ascontiguousarray(w_out.T).astype(bf)  # [INNER, D]
    vones = np.ones((128, 2 * NCB), dtype=bf)
    xTs = [np.ascontiguousarray(x[b].T).astype(bf) for b in range(B)]
    cTs = [np.ascontiguousarray(context[b].T).astype(bf) for b in range(B)]
    in_maps = []
    for c in range(8):
        b, hp = c // 4, c % 4
        hsl = slice(hp * 128, (hp + 1) * 128)
        in_maps.append({
            "xT": xTs[b],
            "ctxT": cTs[b],
            "wqT": np.ascontiguousarray(wqT[:, hsl]),
            "wkT": np.ascontiguousarray(wkT[:, hsl]),
            "wvT": np.ascontiguousarray(wvT[:, hsl]),
            "woT": np.ascontiguousarray(woT[hsl, :]),
            "vones": vones,
        })
    return in_maps


def kernel(x, context, w_q, w_k, w_v, w_out, b_out):
    x = np.asarray(x, dtype=np.float32)
    context = np.asarray(context, dtype=np.float32)
    w_q = np.asarray(w_q, dtype=np.float32)
    w_k = np.asarray(w_k, dtype=np.float32)
    w_v = np.asarray(w_v, dtype=np.float32)
    w_out = np.asarray(w_out, dtype=np.float32)
    b_out = np.asarray(b_out, dtype=np.float32)

    if "nc" not in _CACHE:
        _CACHE["nc"] = build_nc()
    nc = _CACHE["nc"]

    in_maps = make_in_maps(x, context, w_q, w_k, w_v, w_out)
    res = run_bass_kernel_spmd(nc, in_maps, list(range(8))).results

    out = np.zeros((B, S, D), dtype=np.float32)
    for c in range(8):
        out[c // 4] += np.asarray(res[c]["y"], dtype=np.float32)
    out += b_out
    return out
